# revision 8
# baseline (speedup 1.0000x reference)
"""Trainium2 Bass kernel for nn_MoELayer (moe_routing).

Strategy (V1): data-parallel over tokens. Each of the 8 cores processes
T/8 = 1024 tokens through all 8 experts densely (f32r matmuls), computes
the router + exact top-2 mask on device (split-bf16 compensated matmul
for fp32-accurate logits), applies the 0/1 mask to each expert's output,
and writes its output slice. Host only shards/unshards and combines the
per-core scalar loss partials.

Layout: tokens live on the free axis, H/F on partitions ("transposed"
activations), so fc1 -> silu -> fc2 chain needs no on-chip transposes.
"""

import math
import sys

import numpy as np


def _ensure_path():
    try:
        import concourse  # noqa: F401
    except ImportError:
        sys.path.insert(0, "/opt/trn_rl_repo")


_ensure_path()

import concourse.bass as bass  # noqa: E402
import concourse.mybir as mybir  # noqa: E402
import concourse.tile as tile  # noqa: E402
from concourse import bacc  # noqa: E402
from concourse.bass import ts, ds  # noqa: E402
from concourse.bass_utils import run_bass_kernel_spmd  # noqa: E402
from concourse.masks import make_identity  # noqa: E402

B, S, H, E, F = 4, 2048, 1024, 8, 2048
T = B * S           # 8192 tokens
NCORES = 8
TC = T // NCORES    # 1024 tokens per core
NKH = H // 128      # 8   k-tiles over H
NKF = F // 128      # 16  k-tiles over F
NTT = TC // 128     # 8   token tiles per core
NCH = TC // 512     # 2   512-token chunks per core
AUX_COEF = 0.001
Z_COEF = 0.001

F32 = mybir.dt.float32
F32R = mybir.dt.float32r
BF16 = mybir.dt.bfloat16

_cache = {}


def _build():
    nc = bacc.Bacc("TRN2", target_bir_lowering=False, debug=False,
                   num_devices=NCORES)

    xt = nc.dram_tensor("xt", [H, TC], F32R, kind="ExternalInput")
    rwt = nc.dram_tensor("rwt", [H, E], F32, kind="ExternalInput")
    w1 = nc.dram_tensor("w1", [E, H, F], F32R, kind="ExternalInput")
    b1 = nc.dram_tensor("b1", [E, F], F32, kind="ExternalInput")
    w2 = nc.dram_tensor("w2", [E, F, H], F32R, kind="ExternalInput")
    b2 = nc.dram_tensor("b2", [E, H], F32R, kind="ExternalInput")
    outt = nc.dram_tensor("outt", [H, TC], F32, kind="ExternalOutput")
    lacc = nc.dram_tensor("lacc", [128, 4], F32, kind="ExternalOutput")
    mask_dram = nc.dram_tensor("mask_scratch", [E, TC], F32R)

    xt_r = xt[:].rearrange("(kb p) t -> p kb t", p=128)      # [128, 8, TC]
    rwt_r = rwt[:].rearrange("(kb p) e -> p kb e", p=128)    # [128, 8, E]
    b1_r = b1[:].rearrange("e (fb p) -> p e fb", p=128)      # [128, E, 16]
    outt_r = outt[:].rearrange("(hb p) t -> p hb t", p=128)  # [128, 8, TC]

    with tile.TileContext(nc) as tc:
        with (
            tc.tile_pool(name="main", bufs=1) as mp,
            tc.tile_pool(name="wstream", bufs=2) as wp,
            tc.tile_pool(name="hpool", bufs=1) as hp,
            tc.tile_pool(name="psA", bufs=2, space="PSUM") as psA,
            tc.tile_pool(name="psB", bufs=2, space="PSUM") as psB,
            tc.tile_pool(name="psC", bufs=1, space="PSUM") as psC,
        ):
            # ---------- persistent tiles ----------
            xts = mp.tile([128, NKH, TC], F32R)
            nc.sync.dma_start(out=xts[:], in_=xt_r)
            b1s = mp.tile([128, E, F // 128], F32)
            nc.sync.dma_start(out=b1s[:], in_=b1_r)
            b2s = mp.tile([E, H], F32R)
            nc.sync.dma_start(out=b2s[:], in_=b2[:])
            maskT = mp.tile([E, TC], F32R)
            lacc_sb = mp.tile([128, 4], F32)
            nc.vector.memset(lacc_sb[:], 0.0)
            ident = mp.tile([128, 128], F32)
            make_identity(nc, ident[:])

            # ---------- router ----------
            with (
                tc.tile_pool(name="router", bufs=1) as rp,
                tc.tile_pool(name="psR", bufs=1, space="PSUM") as psR,
            ):
                rws = rp.tile([128, NKH, E], F32)
                nc.sync.dma_start(out=rws[:], in_=rwt_r)
                # split x and rw into bf16 hi/lo for fp32-accurate logits
                xhi = rp.tile([128, NKH, TC], BF16)
                xlo = rp.tile([128, NKH, TC], BF16)
                rhi = rp.tile([128, NKH, E], BF16)
                rlo = rp.tile([128, NKH, E], BF16)
                for kb in range(NKH):
                    lo32 = rp.tile([128, TC], F32, tag="lo32")
                    nc.vector.tensor_copy(xhi[:, kb, :], xts[:, kb, :])
                    nc.vector.tensor_tensor(out=lo32[:], in0=xts[:, kb, :],
                                            in1=xhi[:, kb, :],
                                            op=mybir.AluOpType.subtract)
                    nc.vector.tensor_copy(xlo[:, kb, :], lo32[:])
                    rlo32 = rp.tile([128, E], F32, tag="rlo32")
                    nc.vector.tensor_copy(rhi[:, kb, :], rws[:, kb, :])
                    nc.vector.tensor_tensor(out=rlo32[:], in0=rws[:, kb, :],
                                            in1=rhi[:, kb, :],
                                            op=mybir.AluOpType.subtract)
                    nc.vector.tensor_copy(rlo[:, kb, :], rlo32[:])

                for i in range(NTT):
                    lgps = psR.tile([128, E], F32, tag="lgps")
                    pairs = [(xhi, rhi), (xhi, rlo), (xlo, rhi)]
                    n_mm = NKH * len(pairs)
                    k = 0
                    for kb in range(NKH):
                        for (xa, ra) in pairs:
                            nc.tensor.matmul(
                                lgps[:], xa[:, kb, ts(i, 128)], ra[:, kb, :],
                                start=(k == 0), stop=(k == n_mm - 1))
                            k += 1
                    lg = rp.tile([128, E], F32, tag="lg")
                    nc.vector.tensor_copy(lg[:], lgps[:])
                    mx8 = rp.tile([128, 8], F32, tag="mx8")
                    nc.vector.max(out=mx8[:], in_=lg[:])
                    mask = rp.tile([128, E], F32, tag="mask")
                    nc.vector.tensor_scalar(
                        out=mask[:], in0=lg[:], scalar1=mx8[:, 1:2],
                        scalar2=None, op0=mybir.AluOpType.is_ge)
                    mtps = psR.tile([E, 128], F32, tag="mtps")
                    nc.tensor.transpose(out=mtps[:], in_=mask[:],
                                        identity=ident[:])
                    nc.vector.tensor_copy(maskT[:, ts(i, 128)], mtps[:])
                    # loss partials
                    negm = rp.tile([128, 1], F32, tag="negm")
                    nc.vector.tensor_scalar_mul(negm[:], mx8[:, 0:1], -1.0)
                    ex = rp.tile([128, E], F32, tag="ex")
                    sume = rp.tile([128, 1], F32, tag="sume")
                    nc.scalar.activation(ex[:], lg[:],
                                         mybir.ActivationFunctionType.Exp,
                                         bias=negm[:, 0:1], scale=1.0,
                                         accum_out=sume[:])
                    lse = rp.tile([128, 1], F32, tag="lse")
                    nc.scalar.activation(lse[:], sume[:],
                                         mybir.ActivationFunctionType.Ln)
                    nc.vector.tensor_tensor(out=lse[:], in0=lse[:],
                                            in1=mx8[:, 0:1],
                                            op=mybir.AluOpType.add)
                    sq = rp.tile([128, E], F32, tag="sq")
                    sqs = rp.tile([128, 1], F32, tag="sqs")
                    nc.scalar.activation(sq[:], lg[:],
                                         mybir.ActivationFunctionType.Square,
                                         accum_out=sqs[:])
                    lgs = rp.tile([128, 1], F32, tag="lgs")
                    nc.vector.tensor_reduce(out=lgs[:], in_=lg[:],
                                            axis=mybir.AxisListType.X,
                                            op=mybir.AluOpType.add)
                    for col, val in ((0, lse), (1, lgs), (2, sqs)):
                        nc.vector.tensor_tensor(
                            out=lacc_sb[:, col:col + 1],
                            in0=lacc_sb[:, col:col + 1], in1=val[:],
                            op=mybir.AluOpType.add)

            # bounce mask through DRAM so it can be partition-broadcast
            nc.sync.dma_start(out=mask_dram[:], in_=maskT[:])

            # ---------- dense experts ----------
            for c in range(NCH):
                acc = mp.tile([128, NKH, 512], F32, tag="acc")
                for e in range(E):
                    mb = wp.tile([128, 512], F32R, tag="mb")
                    nc.sync.dma_start(
                        out=mb[:],
                        in_=mask_dram[:][e:e + 1, ts(c, 512)].to_broadcast(
                            [128, 512]))
                    h_sb = hp.tile([128, NKF, 512], F32R, tag="h")
                    for quart in range(4):
                        w1h = wp.tile([128, NKH, 512], F32R, tag="w1h")
                        nc.sync.dma_start(
                            out=w1h[:],
                            in_=w1[e].rearrange("(kb p) f -> p kb f", p=128)[
                                :, :, ds(quart * 512, 512)])
                        for fbl in range(4):
                            fb = quart * 4 + fbl
                            zps = psA.tile([128, 512], F32, tag="zps")
                            for kb in range(NKH):
                                nc.tensor.matmul(
                                    zps[:], w1h[:, kb, ts(fbl, 128)],
                                    xts[:, kb, ts(c, 512)],
                                    start=(kb == 0), stop=(kb == NKH - 1))
                            nc.scalar.activation(
                                h_sb[:, fb, :], zps[:],
                                mybir.ActivationFunctionType.Silu,
                                bias=b1s[:, e, fb:fb + 1], scale=1.0)
                    for hb in range(NKH):
                        w2t = wp.tile([128, NKF, 128], F32R, tag="w2t")
                        nc.sync.dma_start(
                            out=w2t[:],
                            in_=w2[e].rearrange("(kf p) hh -> p kf hh", p=128)[
                                :, :, ts(hb, 128)])
                        yps = psB.tile([128, 512], F32, tag="yps")
                        for kf in range(NKF):
                            nc.tensor.matmul(
                                yps[:], w2t[:, kf, :], h_sb[:, kf, :],
                                start=(kf == 0), stop=(kf == NKF - 1))
                        if e == 0:
                            nc.vector.tensor_tensor(
                                out=acc[:, hb, :], in0=yps[:], in1=mb[:],
                                op=mybir.AluOpType.mult)
                        else:
                            tmp = wp.tile([128, 512], F32, tag="tmpy")
                            nc.vector.tensor_tensor(
                                out=tmp[:], in0=yps[:], in1=mb[:],
                                op=mybir.AluOpType.mult)
                            nc.vector.tensor_tensor(
                                out=acc[:, hb, :], in0=acc[:, hb, :],
                                in1=tmp[:], op=mybir.AluOpType.add)
                # bias2 * mask term and writeout
                for hb in range(NKH):
                    b2ps = psC.tile([128, 512], F32, tag="b2ps")
                    nc.tensor.matmul(b2ps[:], b2s[:, ts(hb, 128)],
                                     maskT[:, ts(c, 512)],
                                     start=True, stop=True)
                    outsb = wp.tile([128, 512], F32, tag="outsb")
                    nc.vector.tensor_tensor(out=outsb[:], in0=acc[:, hb, :],
                                            in1=b2ps[:],
                                            op=mybir.AluOpType.add)
                    nc.sync.dma_start(out=outt_r[:, hb, ts(c, 512)],
                                      in_=outsb[:])

            nc.sync.dma_start(out=lacc[:], in_=lacc_sb[:])

    nc.compile()
    return nc


def get_nc():
    if "nc" not in _cache:
        _cache["nc"] = _build()
    return _cache["nc"]


def make_in_maps(hidden_states, router_w, w1, b1, w2, b2):
    x = np.asarray(hidden_states, dtype=np.float32).reshape(T, H)
    rwt = np.ascontiguousarray(np.asarray(router_w, np.float32).T)
    w1 = np.asarray(w1, np.float32)
    b1 = np.asarray(b1, np.float32)
    w2 = np.asarray(w2, np.float32)
    b2 = np.asarray(b2, np.float32)
    in_maps = []
    for c in range(NCORES):
        xt_c = np.ascontiguousarray(x[c * TC:(c + 1) * TC].T)
        in_maps.append({"xt": xt_c, "rwt": rwt, "w1": w1, "b1": b1,
                        "w2": w2, "b2": b2})
    return in_maps


def finalize(results):
    """Combine per-core outputs into (out, router_loss)."""
    out = np.concatenate([r["outt"].T for r in results], axis=0)
    out = np.ascontiguousarray(out).reshape(B, S, H)
    part = np.stack([r["lacc"] for r in results]).sum(axis=(0, 1))
    sum_lse, sum_logits, sum_sq = float(part[0]), float(part[1]), float(part[2])
    sum_logp = sum_logits - E * sum_lse
    aux = (T * math.log(1.0 / E) - sum_logp / E) / B * AUX_COEF
    z = sum_sq / (T * E) * Z_COEF
    return out, np.float32(aux + z)


def kernel(hidden_states, router_w, w1, b1, w2, b2, num_experts_per_tok=2):
    nc = get_nc()
    in_maps = make_in_maps(hidden_states, router_w, w1, b1, w2, b2)
    res = run_bass_kernel_spmd(nc, in_maps, core_ids=list(range(NCORES)))
    return finalize(res.results)


# revision 10
# speedup vs baseline: 1.8298x; 1.8298x over previous
"""Trainium2 Bass kernel for nn_MoELayer (moe_routing).

Strategy (V1): data-parallel over tokens. Each of the 8 cores processes
T/8 = 1024 tokens through all 8 experts densely (f32r matmuls), computes
the router + exact top-2 mask on device (split-bf16 compensated matmul
for fp32-accurate logits), applies the 0/1 mask to each expert's output,
and writes its output slice. Host only shards/unshards and combines the
per-core scalar loss partials.

Layout: tokens live on the free axis, H/F on partitions ("transposed"
activations), so fc1 -> silu -> fc2 chain needs no on-chip transposes.
"""

import math
import sys

import numpy as np


def _ensure_path():
    try:
        import concourse  # noqa: F401
    except ImportError:
        sys.path.insert(0, "/opt/trn_rl_repo")


_ensure_path()

import concourse.bass as bass  # noqa: E402
import concourse.mybir as mybir  # noqa: E402
import concourse.tile as tile  # noqa: E402
from concourse import bacc  # noqa: E402
from concourse.bass import ts, ds  # noqa: E402
from concourse.bass_utils import run_bass_kernel_spmd  # noqa: E402
from concourse.masks import make_identity  # noqa: E402

B, S, H, E, F = 4, 2048, 1024, 8, 2048
T = B * S           # 8192 tokens
NCORES = 8
TC = T // NCORES    # 1024 tokens per core
NKH = H // 128      # 8   k-tiles over H
NKF = F // 128      # 16  k-tiles over F
NTT = TC // 128     # 8   token tiles per core
NCH = TC // 512     # 2   512-token chunks per core
AUX_COEF = 0.001
Z_COEF = 0.001

F32 = mybir.dt.float32
F32R = mybir.dt.float32r
BF16 = mybir.dt.bfloat16

_cache = {}


def _build():
    nc = bacc.Bacc("TRN2", target_bir_lowering=False, debug=False,
                   num_devices=NCORES)

    xt = nc.dram_tensor("xt", [H, TC], F32R, kind="ExternalInput")
    xn = nc.dram_tensor("xn", [TC, H], F32, kind="ExternalInput")
    rw = nc.dram_tensor("rw", [E, H], F32, kind="ExternalInput")
    w1 = nc.dram_tensor("w1", [E, H, F], F32R, kind="ExternalInput")
    b1 = nc.dram_tensor("b1", [E, F], F32, kind="ExternalInput")
    w2 = nc.dram_tensor("w2", [E, F, H], F32R, kind="ExternalInput")
    b2 = nc.dram_tensor("b2", [E, H], F32R, kind="ExternalInput")
    outt = nc.dram_tensor("outt", [H, TC], F32, kind="ExternalOutput")
    lacc = nc.dram_tensor("lacc", [128, 4], F32, kind="ExternalOutput")
    mask_dram = nc.dram_tensor("mask_scratch", [E, TC], F32R)

    xt_r = xt[:].rearrange("(kb p) t -> p kb t", p=128)      # [128, 8, TC]
    xn_r = xn[:].rearrange("(tt p) h -> p tt h", p=128)      # [128, 8, H]
    b1_r = b1[:].rearrange("e (fb p) -> p e fb", p=128)      # [128, E, 16]
    outt_r = outt[:].rearrange("(hb p) t -> p hb t", p=128)  # [128, 8, TC]

    with tile.TileContext(nc) as tc:
        with (
            tc.tile_pool(name="main", bufs=1) as mp,
            tc.tile_pool(name="wstream", bufs=2) as wp,
            tc.tile_pool(name="hpool", bufs=1) as hp,
            tc.tile_pool(name="psA", bufs=2, space="PSUM") as psA,
            tc.tile_pool(name="psB", bufs=2, space="PSUM") as psB,
            tc.tile_pool(name="psC", bufs=1, space="PSUM") as psC,
        ):
            # ---------- persistent tiles ----------
            xts = mp.tile([128, NKH, TC], F32R)
            nc.sync.dma_start(out=xts[:], in_=xt_r)
            b1s = mp.tile([128, E, F // 128], F32)
            nc.sync.dma_start(out=b1s[:], in_=b1_r)
            b2s = mp.tile([E, H], F32R)
            nc.sync.dma_start(out=b2s[:], in_=b2[:])
            maskT = mp.tile([E, TC], F32R)
            lacc_sb = mp.tile([128, 4], F32)
            nc.vector.memset(lacc_sb[:], 0.0)
            ident = mp.tile([128, 128], F32)
            make_identity(nc, ident[:])

            # ---------- router ----------
            with (
                tc.tile_pool(name="router", bufs=1) as rp,
                tc.tile_pool(name="psR", bufs=1, space="PSUM") as psR,
            ):
                xns = rp.tile([128, NTT, H], F32)
                nc.sync.dma_start(out=xns[:], in_=xn_r)
                lg_all = rp.tile([128, NTT * E], F32)
                for e in range(E):
                    rwb = rp.tile([128, H], F32, tag="rwb")
                    nc.sync.dma_start(
                        out=rwb[:],
                        in_=rw[:][e:e + 1, :].to_broadcast([128, H]))
                    for tt in range(NTT):
                        scr = rp.tile([128, H], F32, tag="scr")
                        nc.vector.tensor_tensor(
                            out=scr[:], in0=xns[:, tt, :], in1=rwb[:],
                            op=mybir.AluOpType.mult)
                        nc.vector.tensor_reduce(
                            out=lg_all[:, ts(tt * E + e, 1)], in_=scr[:],
                            axis=mybir.AxisListType.X,
                            op=mybir.AluOpType.add)

                for i in range(NTT):
                    lg = lg_all[:, ds(i * E, E)]
                    mx8 = rp.tile([128, 8], F32, tag="mx8")
                    nc.vector.max(out=mx8[:], in_=lg)
                    mask = rp.tile([128, E], F32, tag="mask")
                    nc.vector.tensor_scalar(
                        out=mask[:], in0=lg, scalar1=mx8[:, 1:2],
                        scalar2=None, op0=mybir.AluOpType.is_ge)
                    mtps = psR.tile([E, 128], F32, tag="mtps")
                    nc.tensor.transpose(out=mtps[:], in_=mask[:],
                                        identity=ident[:])
                    nc.vector.tensor_copy(maskT[:, ts(i, 128)], mtps[:])
                    # loss partials
                    negm = rp.tile([128, 1], F32, tag="negm")
                    nc.vector.tensor_scalar_mul(negm[:], mx8[:, 0:1], -1.0)
                    ex = rp.tile([128, E], F32, tag="ex")
                    sume = rp.tile([128, 1], F32, tag="sume")
                    nc.scalar.activation(ex[:], lg,
                                         mybir.ActivationFunctionType.Exp,
                                         bias=negm[:, 0:1], scale=1.0,
                                         accum_out=sume[:])
                    lse = rp.tile([128, 1], F32, tag="lse")
                    nc.scalar.activation(lse[:], sume[:],
                                         mybir.ActivationFunctionType.Ln)
                    nc.vector.tensor_tensor(out=lse[:], in0=lse[:],
                                            in1=mx8[:, 0:1],
                                            op=mybir.AluOpType.add)
                    sq = rp.tile([128, E], F32, tag="sq")
                    sqs = rp.tile([128, 1], F32, tag="sqs")
                    nc.scalar.activation(sq[:], lg,
                                         mybir.ActivationFunctionType.Square,
                                         accum_out=sqs[:])
                    lgs = rp.tile([128, 1], F32, tag="lgs")
                    nc.vector.tensor_reduce(out=lgs[:], in_=lg,
                                            axis=mybir.AxisListType.X,
                                            op=mybir.AluOpType.add)
                    for col, val in ((0, lse), (1, lgs), (2, sqs)):
                        nc.vector.tensor_tensor(
                            out=lacc_sb[:, col:col + 1],
                            in0=lacc_sb[:, col:col + 1], in1=val[:],
                            op=mybir.AluOpType.add)

            # bounce mask through DRAM so it can be partition-broadcast
            nc.sync.dma_start(out=mask_dram[:], in_=maskT[:])

            # ---------- dense experts ----------
            for c in range(NCH):
                acc = mp.tile([128, NKH, 512], F32, tag="acc")
                for e in range(E):
                    mb = wp.tile([128, 512], F32R, tag="mb")
                    nc.sync.dma_start(
                        out=mb[:],
                        in_=mask_dram[:][e:e + 1, ts(c, 512)].to_broadcast(
                            [128, 512]))
                    h_sb = hp.tile([128, NKF, 512], F32R, tag="h")
                    for quart in range(4):
                        w1h = wp.tile([128, NKH, 512], F32R, tag="w1h")
                        nc.sync.dma_start(
                            out=w1h[:],
                            in_=w1[e].rearrange("(kb p) f -> p kb f", p=128)[
                                :, :, ds(quart * 512, 512)])
                        for fbl in range(4):
                            fb = quart * 4 + fbl
                            zps = psA.tile([128, 512], F32, tag="zps")
                            for kb in range(NKH):
                                nc.tensor.matmul(
                                    zps[:], w1h[:, kb, ts(fbl, 128)],
                                    xts[:, kb, ts(c, 512)],
                                    start=(kb == 0), stop=(kb == NKH - 1))
                            nc.scalar.activation(
                                h_sb[:, fb, :], zps[:],
                                mybir.ActivationFunctionType.Silu,
                                bias=b1s[:, e, fb:fb + 1], scale=1.0)
                    for hb in range(NKH):
                        w2t = wp.tile([128, NKF, 128], F32R, tag="w2t")
                        nc.sync.dma_start(
                            out=w2t[:],
                            in_=w2[e].rearrange("(kf p) hh -> p kf hh", p=128)[
                                :, :, ts(hb, 128)])
                        yps = psB.tile([128, 512], F32, tag="yps")
                        for kf in range(NKF):
                            nc.tensor.matmul(
                                yps[:], w2t[:, kf, :], h_sb[:, kf, :],
                                start=(kf == 0), stop=(kf == NKF - 1))
                        if e == 0:
                            nc.vector.tensor_tensor(
                                out=acc[:, hb, :], in0=yps[:], in1=mb[:],
                                op=mybir.AluOpType.mult)
                        else:
                            tmp = wp.tile([128, 512], F32, tag="tmpy")
                            nc.vector.tensor_tensor(
                                out=tmp[:], in0=yps[:], in1=mb[:],
                                op=mybir.AluOpType.mult)
                            nc.vector.tensor_tensor(
                                out=acc[:, hb, :], in0=acc[:, hb, :],
                                in1=tmp[:], op=mybir.AluOpType.add)
                # bias2 * mask term and writeout
                for hb in range(NKH):
                    b2ps = psC.tile([128, 512], F32, tag="b2ps")
                    nc.tensor.matmul(b2ps[:], b2s[:, ts(hb, 128)],
                                     maskT[:, ts(c, 512)],
                                     start=True, stop=True)
                    outsb = wp.tile([128, 512], F32, tag="outsb")
                    nc.vector.tensor_tensor(out=outsb[:], in0=acc[:, hb, :],
                                            in1=b2ps[:],
                                            op=mybir.AluOpType.add)
                    nc.sync.dma_start(out=outt_r[:, hb, ts(c, 512)],
                                      in_=outsb[:])

            nc.sync.dma_start(out=lacc[:], in_=lacc_sb[:])

    nc.compile()
    return nc


def get_nc():
    if "nc" not in _cache:
        _cache["nc"] = _build()
    return _cache["nc"]


def make_in_maps(hidden_states, router_w, w1, b1, w2, b2):
    x = np.asarray(hidden_states, dtype=np.float32).reshape(T, H)
    rw = np.ascontiguousarray(np.asarray(router_w, np.float32))
    w1 = np.asarray(w1, np.float32)
    b1 = np.asarray(b1, np.float32)
    w2 = np.asarray(w2, np.float32)
    b2 = np.asarray(b2, np.float32)
    in_maps = []
    for c in range(NCORES):
        xn_c = np.ascontiguousarray(x[c * TC:(c + 1) * TC])
        xt_c = np.ascontiguousarray(xn_c.T)
        in_maps.append({"xt": xt_c, "xn": xn_c, "rw": rw, "w1": w1, "b1": b1,
                        "w2": w2, "b2": b2})
    return in_maps


def finalize(results):
    """Combine per-core outputs into (out, router_loss)."""
    out = np.concatenate([r["outt"].T for r in results], axis=0)
    out = np.ascontiguousarray(out).reshape(B, S, H)
    part = np.stack([r["lacc"] for r in results]).sum(axis=(0, 1))
    sum_lse, sum_logits, sum_sq = float(part[0]), float(part[1]), float(part[2])
    sum_logp = sum_logits - E * sum_lse
    aux = (T * math.log(1.0 / E) - sum_logp / E) / B * AUX_COEF
    z = sum_sq / (T * E) * Z_COEF
    return out, np.float32(aux + z)


def kernel(hidden_states, router_w, w1, b1, w2, b2, num_experts_per_tok=2):
    nc = get_nc()
    in_maps = make_in_maps(hidden_states, router_w, w1, b1, w2, b2)
    res = run_bass_kernel_spmd(nc, in_maps, core_ids=list(range(NCORES)))
    return finalize(res.results)
